# revision 5
# baseline (speedup 1.0000x reference)
"""Trainium2 Bass kernel for nn_DenseHypercube (encoder + DomainBounder + 625-entry lookup).

Pipeline per NeuronCore (data-parallel over 8 cores, 262144 samples each):
  phase A (64 rounds x 4096 samples):
    DMA x round -> SBUF [128, 32 slots, 32 feat]   (partition p owns samples [p*2048,(p+1)*2048))
    8x PE transpose -> PSUM xT tiles [32q+f, cols]
    DVE copy PSUM->SBUF xT [128, 1024]
    mm1 (blockdiag W1 x4, K=128) -> fused bank rows 0:80
    mm2 (blockdiag W2 x4, K=80, data r-1) -> rows 96:116 (accumulate over mm3's zeros)
    mm3 (blockdiag W3 x4 padded M=32, K=20, data r-2, start=True) -> rows 96:128
    one fused ACT tanh+bias [128, 1024] PSUM->SBUF
    e-compact DMAs: e rows 120:128 -> E0/E1 [128, 2048]
  phase B: DVE min/max reduce, PE transpose, cross-core AllReduce(max) of [mx0,mx1,-mn0,-mn1]
  phase C: affine+floor (RNE(x-0.5)) -> bucket idx -> flat = 25*i0+i1 -> int16
  gather:  8x gpsimd ap_gather from the 625-entry table; DMA out in 1KB runs.
"""
import sys
sys.path.insert(0, "/opt/trn_rl_repo")
import numpy as np
import concourse.bass as bass
import concourse.bacc as bacc
import concourse.tile as tile
from concourse import mybir
from concourse.bass_utils import run_bass_kernel_spmd

F32 = mybir.dt.float32
I32 = mybir.dt.int32
I16 = mybir.dt.int16

NCORES = 8
N_TOT = 2097152
D_IN = 32
PER_CORE = N_TOT // NCORES          # 262144
PER_PART = PER_CORE // 128          # 2048 samples per partition
ROUNDS = 64                         # rounds of 4096 samples
SLOTS = 32                          # samples per partition per round
N_DIV = 25
MARG_LO, MARG_HI = 0.01, 0.99
DIV_W = 1.0 / N_DIV

_CACHE = {}


def _build(reps=1):
    nc = bacc.Bacc(None, num_devices=NCORES)
    x_h = nc.declare_dram_parameter("x", [PER_CORE, D_IN], F32, isOutput=False)
    bd1_h = nc.declare_dram_parameter("bd1", [128, 80], F32, isOutput=False)
    bd2_h = nc.declare_dram_parameter("bd2", [80, 20], F32, isOutput=False)
    bd3_h = nc.declare_dram_parameter("bd3", [20, 32], F32, isOutput=False)
    bias_h = nc.declare_dram_parameter("biasv", [128, 1], F32, isOutput=False)
    id_h = nc.declare_dram_parameter("ident", [128, 128], F32, isOutput=False)
    tb_h = nc.declare_dram_parameter("bm", [1, 625], F32, isOutput=False)
    y_h = nc.declare_dram_parameter("y", [1, PER_CORE], F32, isOutput=True)

    with tile.TileContext(nc) as tc:
        with (
            tc.tile_pool(name="const", bufs=1) as const,
            tc.tile_pool(name="xp", bufs=2) as xp,
            tc.tile_pool(name="xtp", bufs=2) as xtp,
            tc.tile_pool(name="actp", bufs=3) as actp,
            tc.tile_pool(name="ep", bufs=1) as ep,
            tc.tile_pool(name="cp", bufs=1) as cp,
            tc.tile_pool(name="gp", bufs=2) as gp,
            tc.tile_pool(name="ps_t", bufs=2, space="PSUM") as ps_t,
            tc.tile_pool(name="ps_f", bufs=2, space="PSUM") as ps_f,
            tc.tile_pool(name="dram", bufs=1, space="DRAM") as dram,
        ):
            # ---------------- constants ----------------
            bd1_sb = const.tile([128, 80], F32)
            nc.sync.dma_start(out=bd1_sb, in_=bd1_h[:, :])
            bd2_sb = const.tile([80, 20], F32)
            nc.sync.dma_start(out=bd2_sb, in_=bd2_h[:, :])
            bd3_sb = const.tile([128, 32], F32)
            nc.sync.dma_start(out=bd3_sb[96:116, :], in_=bd3_h[:, :])
            bias_sb = const.tile([128, 1], F32)
            nc.sync.dma_start(out=bias_sb, in_=bias_h[:, :])
            id_sb = const.tile([128, 128], F32)
            nc.sync.dma_start(out=id_sb, in_=id_h[:, :])
            tb_sb = const.tile([128, 625], F32)
            nc.sync.dma_start(out=tb_sb, in_=bass.AP(
                tensor=tb_h.ap().tensor, offset=0, ap=[[0, 128], [1, 625]]))

            for rep in range(reps):
                # ---------------- phase A ----------------
                act_prev = None
                e_tiles = []
                E0 = ep.tile([128, PER_PART], F32, name="E0", tag="E0")
                E1 = ep.tile([128, PER_PART], F32, name="E1", tag="E1")
                E = (E0, E1)
                for r in range(ROUNDS + 2):
                    if r < ROUNDS:
                        x_nat = xp.tile([128, SLOTS * D_IN], F32, name="x_nat",
                                        tag="x_nat")
                        nc.sync.dma_start(out=x_nat, in_=bass.AP(
                            tensor=x_h.ap().tensor,
                            offset=r * SLOTS * D_IN,
                            ap=[[PER_PART * D_IN, 128], [1, SLOTS * D_IN]]))
                        pt = ps_t.tile([128, 1024], F32, name="pt", tag="pt")
                        for t in range(8):
                            nc.tensor.transpose(
                                pt[:, 128 * t:128 * (t + 1)],
                                x_nat[:, 128 * t:128 * (t + 1)],
                                id_sb)
                        xT = xtp.tile([128, 1024], F32, name="xT", tag="xT")
                        nc.vector.tensor_copy(xT, pt)

                    fb = ps_f.tile([128, 1024], F32, name="fb", tag="fb")
                    if r >= 2:
                        for h in (0, 1):
                            nc.tensor.matmul(
                                fb[96:128, 512 * h:512 * (h + 1)],
                                bd3_sb[96:116, :],
                                act_prev[96:116, 512 * h:512 * (h + 1)],
                                start=True, stop=(r == ROUNDS + 1),
                                tile_position=(96, 96))
                    if 1 <= r <= ROUNDS:
                        for h in (0, 1):
                            nc.tensor.matmul(
                                fb[96:116, 512 * h:512 * (h + 1)],
                                bd2_sb,
                                act_prev[0:80, 512 * h:512 * (h + 1)],
                                start=(r == 1), stop=True,
                                tile_position=(0, 96))
                    if r < ROUNDS:
                        for h in (0, 1):
                            nc.tensor.matmul(
                                fb[0:80, 512 * h:512 * (h + 1)],
                                bd1_sb,
                                xT[:, 512 * h:512 * (h + 1)],
                                start=True, stop=True)

                    act_out = actp.tile([128, 1024], F32, name="act_out",
                                        tag="act_out")
                    nc.scalar.activation(act_out, fb,
                                         mybir.ActivationFunctionType.Tanh,
                                         bias=bias_sb[:, 0:1], scale=1.0)

                    if r >= 2:
                        rd = r - 2
                        av = act_out.rearrange("p (h t c) -> p h t c", h=2, t=4)
                        for d in (0, 1):
                            ev = E[d].rearrange("p (r h c) -> p r h c", r=8, h=2)
                            for tp_ in range(4):
                                pb = (rd // 8) * 16 + 4 * tp_
                                nc.sync.dma_start(
                                    out=ev[pb:pb + 4, rd % 8, :, :],
                                    in_=av[120 + d:128:2, :, tp_, :])
                    act_prev = act_out

                # ---------------- phase B: min/max + collective ----------------
                pk = cp.tile([128, 4], F32, name="pk", tag="pk")
                AX = mybir.AxisListType.X
                nc.vector.tensor_reduce(pk[:, 0:1], E0, axis=AX,
                                        op=mybir.AluOpType.max)
                nc.vector.tensor_reduce(pk[:, 1:2], E1, axis=AX,
                                        op=mybir.AluOpType.max)
                nc.vector.tensor_reduce(pk[:, 2:3], E0, axis=AX,
                                        op=mybir.AluOpType.min)
                nc.vector.tensor_reduce(pk[:, 3:4], E1, axis=AX,
                                        op=mybir.AluOpType.min)
                nc.vector.tensor_scalar(pk[:, 2:4], pk[:, 2:4], -1.0, None,
                                        mybir.AluOpType.mult)
                pt4 = ps_f.tile([4, 128], F32, name="pt4", tag="fb")
                nc.tensor.transpose(pt4, pk, id_sb)
                g4 = cp.tile([4, 1], F32, name="g4", tag="g4")
                nc.vector.tensor_reduce(g4, pt4, axis=AX,
                                        op=mybir.AluOpType.max)
                pg = ps_f.tile([1, 4], F32, name="pg", tag="fb")
                nc.tensor.transpose(pg, g4, id_sb[0:4, 0:4])
                gl_sb = cp.tile([1, 4], F32, name="gl_sb", tag="gl_sb")
                nc.vector.tensor_copy(gl_sb, pg)

                cc_in = dram.tile([1, 4], F32, name="cc_in", tag="cc_in")
                cc_out = dram.tile([1, 4], F32, addr_space="Shared",
                                   name="cc_out", tag="cc_out")
                nc.sync.dma_start(out=cc_in[:], in_=gl_sb)
                nc.gpsimd.collective_compute(
                    "AllReduce", mybir.AluOpType.max,
                    replica_groups=[list(range(NCORES))],
                    ins=[cc_in.opt()], outs=[cc_out.opt()])
                gg = cp.tile([1, 4], F32, name="gg", tag="gg")
                nc.sync.dma_start(out=gg, in_=cc_out[:])

                # scalars: A = 24.5/(mx-mn); B'' = -0.25 + negmn*A
                sc = cp.tile([1, 8], F32, name="sc", tag="sc")
                # sc[0:2] = D = mx + negmn
                nc.vector.tensor_tensor(out=sc[:, 0:2], in0=gg[:, 0:2],
                                        in1=gg[:, 2:4], op=mybir.AluOpType.add)
                # sc[2:4] = 1/D
                nc.vector.reciprocal(sc[:, 2:4], sc[:, 0:2])
                # sc[4:6] = A = 24.5/D
                nc.vector.tensor_scalar(sc[:, 4:6], sc[:, 2:4], 24.5, None,
                                        mybir.AluOpType.mult)
                # sc[6:8] = B'' = negmn*A - 0.25
                nc.vector.tensor_tensor(out=sc[:, 6:8], in0=gg[:, 2:4],
                                        in1=sc[:, 4:6], op=mybir.AluOpType.mult)
                nc.vector.tensor_scalar(sc[:, 6:8], sc[:, 6:8], -0.25, None,
                                        mybir.AluOpType.add)
                # broadcast A0,A1,B0,B1 to [128,1] via DRAM bounce
                sd = dram.tile([1, 8], F32, name="sd", tag="sd")
                nc.sync.dma_start(out=sd[:], in_=sc)
                ab = cp.tile([128, 4], F32, name="ab", tag="ab")
                for i, col in enumerate((4, 5, 6, 7)):
                    nc.sync.dma_start(out=ab[:, i:i + 1], in_=bass.AP(
                        tensor=sd.tensor, offset=col, ap=[[0, 128], [1, 1]]))

                # ---------------- phase C: bucket indices ----------------
                f0 = cp.tile([128, PER_PART], F32, name="f0", tag="f0")
                i0 = cp.tile([128, PER_PART], I32, name="i0", tag="i0")
                i0f = cp.tile([128, PER_PART], F32, name="i0f", tag="i0f")
                i1f = cp.tile([128, PER_PART], F32, name="i1f", tag="i1f")
                flat16 = cp.tile([128, PER_PART], I16, name="flat16", tag="fl")
                idxg = cp.tile([128, PER_PART], I16, name="idxg", tag="ix")
                for d in (0, 1):
                    nc.vector.tensor_scalar(f0, E[d], ab[:, d:d + 1],
                                            ab[:, 2 + d:3 + d],
                                            mybir.AluOpType.mult,
                                            mybir.AluOpType.add)
                    nc.vector.tensor_copy(i0, f0)            # f32->i32 RNE
                    nc.vector.tensor_copy((i0f, i1f)[d], i0)  # i32->f32 exact
                nc.vector.scalar_tensor_tensor(
                    out=flat16, in0=i0f, scalar=25.0, in1=i1f,
                    op0=mybir.AluOpType.mult, op1=mybir.AluOpType.add)
                # permute free dim: w=(rr*2+h)*128+c  ->  w'=c*16+rr*2+h
                fv = flat16.rearrange("p (r h c) -> p r h c", r=8, h=2)
                src_perm = bass.AP(tensor=fv.tensor, offset=fv.offset,
                                   ap=[[PER_PART, 128], [1, 128], [256, 8],
                                       [128, 2]])
                idxg4 = idxg.rearrange("p (c r h) -> p c r h", c=128, r=8)
                nc.vector.tensor_copy(idxg4, src_perm)

                # ---------------- gather + output ----------------
                for kc in range(8):
                    g_out = gp.tile([128, 4096], F32, name="g_out", tag="g_out")
                    nc.gpsimd.ap_gather(
                        g_out[:, :], tb_sb[:, :],
                        idxg[:, 256 * kc:256 * (kc + 1)],
                        channels=128, num_elems=625, d=1, num_idxs=4096)
                    gv = g_out.rearrange("p (c j) -> p c j", c=16)
                    nc.sync.dma_start(
                        out=bass.AP(tensor=y_h.ap().tensor,
                                    offset=16 * kc * PER_PART,
                                    ap=[[256, 8], [PER_PART, 16], [1, 256]]),
                        in_=gv[0:128:16, :, :])
    nc.compile()
    return nc


def _prep_consts(W1, b1, W2, b2, W3, b3):
    bd1 = np.zeros((128, 80), dtype=np.float32)
    bd2 = np.zeros((80, 20), dtype=np.float32)
    bd3 = np.zeros((20, 32), dtype=np.float32)
    biasv = np.zeros((128, 1), dtype=np.float32)
    for q in range(4):
        # bd1[32q+f, 20q+j] = W1[j, f]
        bd1[32 * q:32 * q + 32, 20 * q:20 * q + 20] = W1.T
        # bd2[20q+j, 5q+k] = W2[k, j]
        bd2[20 * q:20 * q + 20, 5 * q:5 * q + 5] = W2.T
        # bd3[5q+k, 24+2q+d] = W3[d, k]
        bd3[5 * q:5 * q + 5, 24 + 2 * q:24 + 2 * q + 2] = W3.T
        biasv[20 * q:20 * q + 20, 0] = b1
        biasv[96 + 5 * q:96 + 5 * q + 5, 0] = b2
        biasv[120 + 2 * q:120 + 2 * q + 2, 0] = b3
    return bd1, bd2, bd3, biasv


def kernel(x, W1, b1, W2, b2, W3, b3, b_m):
    x = np.ascontiguousarray(np.asarray(x, dtype=np.float32))
    bd1, bd2, bd3, biasv = _prep_consts(
        np.asarray(W1, np.float32), np.asarray(b1, np.float32),
        np.asarray(W2, np.float32), np.asarray(b2, np.float32),
        np.asarray(W3, np.float32), np.asarray(b3, np.float32))
    ident = np.eye(128, dtype=np.float32)
    bm = np.asarray(b_m, np.float32).reshape(1, 625)

    if "nc" not in _CACHE:
        _CACHE["nc"] = _build()
    nc = _CACHE["nc"]

    in_maps = []
    for c in range(NCORES):
        in_maps.append(dict(
            x=x[c * PER_CORE:(c + 1) * PER_CORE],
            bd1=bd1, bd2=bd2, bd3=bd3, biasv=biasv, ident=ident, bm=bm))
    res = run_bass_kernel_spmd(nc, in_maps, core_ids=list(range(NCORES)))
    y = np.concatenate([res.results[c]["y"][0] for c in range(NCORES)])
    return y.reshape(N_TOT, 1)
